# revision 8
# baseline (speedup 1.0000x reference)
# CPAMDec attention decoder kernel for Trainium2 (Bass/Tile), SPMD over 8 cores.
#
# Reference computation (per batch n):
#   q = (Wq @ x_n + bq)            # (C4, HW)   1x1 conv as matmul
#   k = y_n @ Wk.T + bk            # (K, C4)
#   v = y_n @ Wv.T + bv            # (K, C)
#   energy[p,kk] = sum_m q[m,p] k[kk,m]          # (HW, K)
#   att = softmax(energy, axis=-1)
#   out_attn[c,p] = sum_kk v[kk,c] att[p,kk]     # (C, HW)
#   out = scale * out_attn + x
#
# Sharding: data-parallel over N across the 8 cores (1 batch each); weights
# replicated.  Host-side prep: weights are pre-transposed (contraction dim
# first) and `scale` is folded into Wv/bv so the device never needs it.
# bv is applied via a rank-1 PSUM accumulate onto v (ones(1,K).T @ sbv(1,C)),
# which is exact because softmax rows sum to 1.
#
# Per-core pipeline (8 chunks of 512 HW positions):
#   DMA x-chunk -> q = WqT.T @ x (PE, fp32r) -> +bq (ACT) ->
#   energy (PE, fp32) -> exp+rowsum (ACT accum_out) -> 1/sum (DVE) ->
#   att = exp*recip (DVE) -> att^T (PE transpose) -> copy (ACT) ->
#   U = v.T @ att^T (PE, fp32r) -> out = U + x (DVE, PSUM+SBUF->SBUF) -> DMA out
#
# float32r is used only for the two big GEMMs (q-proj, out-bmm): 1 cycle/row
# vs fp32's 4.  Everything feeding the residual path stays fp32.

import numpy as np

import concourse.bacc as bacc
import concourse.mybir as mybir
import concourse.tile as tile
from concourse.bass import ts
from concourse.bass_utils import run_bass_kernel_spmd
from concourse.masks import make_identity

N, C, H, W = 8, 512, 64, 64
HW = H * W          # 4096
K = 64              # gathering centers
C4 = C // 4         # 128
A = C // 128        # 4 c-chunks of 128
CHUNK = 512         # hw positions per pipeline chunk
NJ = HW // CHUNK    # 8
NS = CHUNK // 128   # 4 energy sub-chunks per chunk

F32 = mybir.dt.float32
F32R = mybir.dt.float32r

Ident = mybir.ActivationFunctionType.Identity
Exp = mybir.ActivationFunctionType.Exp


def build(f32r: bool = True):
    nc = bacc.Bacc("TRN2", target_bir_lowering=False, debug=False)

    x_d = nc.dram_tensor("x", [C, HW], F32, kind="ExternalInput").ap()
    yt_d = nc.dram_tensor("yt", [C, K], F32, kind="ExternalInput").ap()
    wqt_d = nc.dram_tensor("wqt", [C, C4], F32, kind="ExternalInput").ap()
    wkt_d = nc.dram_tensor("wkt", [C, C4], F32, kind="ExternalInput").ap()
    wvt_d = nc.dram_tensor("wvt", [C, C], F32, kind="ExternalInput").ap()
    bq_d = nc.dram_tensor("bq", [C4], F32, kind="ExternalInput").ap()
    bk_d = nc.dram_tensor("bk", [C4], F32, kind="ExternalInput").ap()
    sbv_d = nc.dram_tensor("sbv", [C], F32, kind="ExternalInput").ap()
    out_d = nc.dram_tensor("out", [C, HW], F32, kind="ExternalOutput").ap()

    def r(ap):
        # float32r view for matmul operands AND for the instructions that
        # produce them (the BIR verifier requires fp32r-matmul inputs to be
        # written with float32r output dtype).
        return ap.bitcast(F32R) if f32r else ap

    with tile.TileContext(nc) as tc:
        with (
            tc.tile_pool(name="const", bufs=1) as cp,
            tc.tile_pool(name="xin", bufs=3) as xp,
            tc.tile_pool(name="q", bufs=2) as qp,
            tc.tile_pool(name="soft", bufs=2) as sp,
            tc.tile_pool(name="attT", bufs=2) as ap_,
            tc.tile_pool(name="osb", bufs=3) as op_,
            tc.tile_pool(name="ps_q", bufs=2, space="PSUM") as ps_q,
            tc.tile_pool(name="ps_e", bufs=1, space="PSUM") as ps_e,
            tc.tile_pool(name="ps_t", bufs=1, space="PSUM") as ps_t,
            tc.tile_pool(name="ps_o", bufs=4, space="PSUM") as ps_o,
        ):
            # ---------- constants & weights ----------
            wq = cp.tile([128, A, C4], F32)
            nc.sync.dma_start(
                r(wq[:]), r(wqt_d.rearrange("(a p) m -> p a m", p=128))
            )
            wk = cp.tile([128, A, C4], F32)
            nc.sync.dma_start(wk[:], wkt_d.rearrange("(a p) m -> p a m", p=128))
            wv = cp.tile([128, A, C], F32)
            nc.sync.dma_start(wv[:], wvt_d.rearrange("(a p) m -> p a m", p=128))
            yt = cp.tile([128, A, K], F32)
            nc.sync.dma_start(yt[:], yt_d.rearrange("(a p) k -> p a k", p=128))
            bq_t = cp.tile([C4, 1], F32)
            nc.sync.dma_start(bq_t[:], bq_d.unsqueeze(1))
            bk_t = cp.tile([C4, 1], F32)
            nc.sync.dma_start(bk_t[:], bk_d.unsqueeze(1))
            sbv_row = cp.tile([1, C], F32)
            nc.sync.dma_start(sbv_row[:], sbv_d.unsqueeze(0))

            ones_row = cp.tile([1, K], F32)
            nc.gpsimd.memset(ones_row[:], 1.0)
            ident = cp.tile([128, 128], F32)
            make_identity(nc, ident[:])

            # ---------- k^T = WkT.T @ yT + bk : (C4, K) ----------
            ps_k = ps_q.tile([C4, K], F32, tag="psq")
            for a in range(A):
                nc.tensor.matmul(
                    ps_k[:], wk[:, a, :], yt[:, a, :],
                    start=(a == 0), stop=(a == A - 1),
                )
            kT = cp.tile([C4, K], F32)
            nc.scalar.activation(r(kT[:]), ps_k[:], Ident, bias=bk_t[:])

            # ---------- v_s = yT.T @ WvTs + ones.T @ sbv : (K, C) ----------
            ps_v = ps_t.tile([K, C], F32, tag="pst")
            for a in range(A):
                nc.tensor.matmul(
                    ps_v[:], yt[:, a, :], wv[:, a, :],
                    start=(a == 0), stop=False,
                )
            nc.tensor.matmul(ps_v[:], ones_row[:], sbv_row[:], start=False, stop=True)
            v_sb = cp.tile([K, C], F32)
            nc.vector.tensor_copy(r(v_sb[:]), ps_v[:])

            # ---------- streaming pipeline over HW chunks ----------
            for j in range(NJ):
                xt = xp.tile([128, A, CHUNK], F32, tag="xt")
                nc.sync.dma_start(
                    r(xt[:]),
                    r(x_d[:, ts(j, CHUNK)].rearrange("(a p) q -> p a q", p=128)),
                )

                # q = WqT.T @ x + bq : (C4, CHUNK)
                psq = ps_q.tile([C4, CHUNK], F32, tag="psq")
                for a in range(A):
                    nc.tensor.matmul(
                        psq[:], r(wq[:, a, :]), r(xt[:, a, :]),
                        start=(a == 0), stop=(a == A - 1),
                    )
                q_sb = qp.tile([C4, CHUNK], F32, tag="q")
                nc.scalar.activation(r(q_sb[:]), psq[:], Ident, bias=bq_t[:])

                # energy = q.T @ kT : (CHUNK, K) as NS tiles of (128, K)
                pse = ps_e.tile([128, NS, K], F32, tag="pse")
                for s in range(NS):
                    nc.tensor.matmul(
                        pse[:, s, :], r(q_sb[:, ts(s, 128)]), r(kT[:]),
                        start=True, stop=True,
                    )

                # softmax over K (free axis); no max-subtraction needed:
                # |energy| <~ 25 so exp stays well inside fp32 range.
                exp_sb = sp.tile([128, NS, K], F32, tag="exp")
                sums = sp.tile([128, NS], F32, tag="sums")
                for s in range(NS):
                    nc.scalar.activation(
                        exp_sb[:, s, :], pse[:, s, :], Exp,
                        accum_out=sums[:, s : s + 1],
                    )
                recip = sp.tile([128, NS], F32, tag="recip")
                nc.vector.reciprocal(recip[:], sums[:])
                att = sp.tile([128, NS, K], F32, tag="att")
                for s in range(NS):
                    nc.vector.tensor_scalar_mul(
                        att[:, s, :], exp_sb[:, s, :], recip[:, s : s + 1]
                    )

                # att^T via PE transpose: (128, K) -> (K, 128) slices
                pst = ps_t.tile([K, CHUNK], F32, tag="pst")
                for s in range(NS):
                    nc.tensor.transpose(pst[:, ts(s, 128)], att[:, s, :], ident[:])
                attT = ap_.tile([K, CHUNK], F32, tag="attT")
                nc.scalar.copy(r(attT[:]), pst[:])

                # out = v_s.T @ att^T + x : (C, CHUNK)
                outt = op_.tile([128, A, CHUNK], F32, tag="outt")
                for a in range(A):
                    pso = ps_o.tile([128, CHUNK], F32, tag="pso")
                    nc.tensor.matmul(
                        pso[:], r(v_sb[:, ts(a, 128)]), r(attT[:]),
                        start=True, stop=True,
                    )
                    nc.vector.tensor_add(outt[:, a, :], pso[:], xt[:, a, :])
                nc.sync.dma_start(
                    out_d[:, ts(j, CHUNK)].rearrange("(a p) q -> p a q", p=128),
                    outt[:],
                )

    nc.compile()
    return nc


def prep_inputs(x, y, Wq, bq, Wk, bk, Wv, bv, scale):
    """Host-side prep: per-core input maps (weights transposed, scale folded)."""
    x = np.asarray(x, dtype=np.float32)
    y = np.asarray(y, dtype=np.float32)
    s = float(np.asarray(scale).reshape(-1)[0])
    shared = {
        "wqt": np.ascontiguousarray(np.asarray(Wq, np.float32).T),
        "wkt": np.ascontiguousarray(np.asarray(Wk, np.float32).T),
        "wvt": np.ascontiguousarray(np.asarray(Wv, np.float32).T * s),
        "bq": np.ascontiguousarray(np.asarray(bq, np.float32)),
        "bk": np.ascontiguousarray(np.asarray(bk, np.float32)),
        "sbv": np.ascontiguousarray(np.asarray(bv, np.float32) * s),
    }
    in_maps = []
    for n in range(N):
        in_maps.append(
            {
                "x": np.ascontiguousarray(x[n].reshape(C, HW)),
                "yt": np.ascontiguousarray(y[n].T),
                **shared,
            }
        )
    return in_maps


_NC_CACHE = {}


def get_nc(f32r: bool = True):
    if f32r not in _NC_CACHE:
        _NC_CACHE[f32r] = build(f32r)
    return _NC_CACHE[f32r]


def kernel(x, y, Wq, bq, Wk, bk, Wv, bv, scale, **run_kwargs):
    nc = get_nc()
    in_maps = prep_inputs(x, y, Wq, bq, Wk, bk, Wv, bv, scale)
    res = run_bass_kernel_spmd(nc, in_maps, core_ids=list(range(N)), **run_kwargs)
    out = np.stack([res.results[n]["out"] for n in range(N)], axis=0)
    return out.reshape(N, C, H, W).astype(np.float32)


# revision 9
# speedup vs baseline: 1.0437x; 1.0437x over previous
# CPAMDec attention decoder kernel for Trainium2 (Bass/Tile), SPMD over 8 cores.
#
# Reference computation (per batch n):
#   q = (Wq @ x_n + bq)            # (C4, HW)   1x1 conv as matmul
#   k = y_n @ Wk.T + bk            # (K, C4)
#   v = y_n @ Wv.T + bv            # (K, C)
#   energy[p,kk] = sum_m q[m,p] k[kk,m]          # (HW, K)
#   att = softmax(energy, axis=-1)
#   out_attn[c,p] = sum_kk v[kk,c] att[p,kk]     # (C, HW)
#   out = scale * out_attn + x
#
# Sharding: data-parallel over N across the 8 cores (1 batch each); weights
# replicated.  Host-side prep: weights are pre-transposed (contraction dim
# first) and `scale` is folded into Wv/bv so the device never needs it.
# bv is applied via a rank-1 PSUM accumulate onto v (ones(1,K).T @ sbv(1,C)),
# which is exact because softmax rows sum to 1.
#
# Per-core pipeline: x streams in 4 DMA chunks of (C, 1024) (4KB descriptors),
# compute runs on 8 sub-chunks of 512 positions:
#   q = WqT.T @ x (PE, fp32r) -> +bq (ACT) -> energy (PE, fp32r) ->
#   exp+rowsum (ACT accum_out) -> 1/sum (DVE) -> att = exp*recip (ACT scale) ->
#   att^T (PE transpose) -> copy (ACT) -> U = v.T @ att^T (PE, fp32r) ->
#   out = U + x (DVE, PSUM+SBUF->SBUF) -> DMA out per 1024-chunk
#
# float32r (1 cycle/row vs fp32's 4) is used for q-proj, energy and out-bmm;
# the residual path (x, +) stays exact fp32.  The BIR verifier requires fp32r
# matmul inputs to be *produced* as float32r, hence the bitcast views on the
# producing instructions (bytes unchanged for DMA; ACT/DVE round on write).

import numpy as np

import concourse.bacc as bacc
import concourse.mybir as mybir
import concourse.tile as tile
from concourse.bass import ts
from concourse.bass_utils import run_bass_kernel_spmd
from concourse.masks import make_identity

N, C, H, W = 8, 512, 64, 64
HW = H * W          # 4096
K = 64              # gathering centers
C4 = C // 4         # 128
A = C // 128        # 4 c-chunks of 128
DCHUNK = 1024       # hw positions per DMA chunk
ND = HW // DCHUNK   # 4
CHUNK = 512         # hw positions per compute chunk
NJ = HW // CHUNK    # 8
NS = CHUNK // 128   # 4 energy sub-chunks per compute chunk

F32 = mybir.dt.float32
F32R = mybir.dt.float32r

Ident = mybir.ActivationFunctionType.Identity
Exp = mybir.ActivationFunctionType.Exp


def build(f32r: bool = True):
    nc = bacc.Bacc("TRN2", target_bir_lowering=False, debug=False)

    x_d = nc.dram_tensor("x", [C, HW], F32, kind="ExternalInput").ap()
    yt_d = nc.dram_tensor("yt", [C, K], F32, kind="ExternalInput").ap()
    wqt_d = nc.dram_tensor("wqt", [C, C4], F32, kind="ExternalInput").ap()
    wkt_d = nc.dram_tensor("wkt", [C, C4], F32, kind="ExternalInput").ap()
    wvt_d = nc.dram_tensor("wvt", [C, C], F32, kind="ExternalInput").ap()
    bq_d = nc.dram_tensor("bq", [C4], F32, kind="ExternalInput").ap()
    bk_d = nc.dram_tensor("bk", [C4], F32, kind="ExternalInput").ap()
    sbv_d = nc.dram_tensor("sbv", [C], F32, kind="ExternalInput").ap()
    out_d = nc.dram_tensor("out", [C, HW], F32, kind="ExternalOutput").ap()

    def r(ap):
        # float32r view for fp32r-matmul operands and their producers.
        return ap.bitcast(F32R) if f32r else ap

    with tile.TileContext(nc) as tc:
        with (
            tc.tile_pool(name="const", bufs=1) as cp,
            tc.tile_pool(name="xin", bufs=3) as xp,
            tc.tile_pool(name="q", bufs=3) as qp,
            tc.tile_pool(name="soft", bufs=3) as sp,
            tc.tile_pool(name="attT", bufs=3) as ap_,
            tc.tile_pool(name="osb", bufs=3) as op_,
            tc.tile_pool(name="ps_q", bufs=2, space="PSUM") as ps_q,
            tc.tile_pool(name="ps_e", bufs=1, space="PSUM") as ps_e,
            tc.tile_pool(name="ps_t", bufs=2, space="PSUM") as ps_t,
            tc.tile_pool(name="ps_o", bufs=3, space="PSUM") as ps_o,
        ):
            # ---- small DMAs first (don't queue them behind MBs of weights)
            bq_t = cp.tile([C4, 1], F32)
            nc.sync.dma_start(bq_t[:], bq_d.unsqueeze(1))
            bk_t = cp.tile([C4, 1], F32)
            nc.sync.dma_start(bk_t[:], bk_d.unsqueeze(1))
            sbv_row = cp.tile([1, C], F32)
            nc.sync.dma_start(sbv_row[:], sbv_d.unsqueeze(0))
            yt = cp.tile([128, A, K], F32)
            nc.sync.dma_start(yt[:], yt_d.rearrange("(a p) k -> p a k", p=128))

            # ---- q-projection weights, then the first x chunk
            wq = cp.tile([128, A, C4], F32)
            nc.sync.dma_start(
                r(wq[:]), r(wqt_d.rearrange("(a p) m -> p a m", p=128))
            )

            xts = []
            def load_x(d):
                xt = xp.tile([128, A, DCHUNK], F32, tag="xt", name=f"xt{d}")
                nc.sync.dma_start(
                    r(xt[:]),
                    r(x_d[:, ts(d, DCHUNK)].rearrange("(a p) q -> p a q", p=128)),
                )
                return xt
            xts.append(load_x(0))

            # pre-trigger the Exp ACT table load so it overlaps the DMAs
            warm = cp.tile([1, 1], F32)
            nc.scalar.activation(warm[:], bq_t[0:1, :], Exp)

            ones_row = cp.tile([1, K], F32)
            nc.gpsimd.memset(ones_row[:], 1.0)
            ident = cp.tile([128, 128], F32)
            make_identity(nc, ident[:])

            wk = cp.tile([128, A, C4], F32)
            nc.sync.dma_start(wk[:], wkt_d.rearrange("(a p) m -> p a m", p=128))
            wv = cp.tile([128, A, C], F32)
            nc.sync.dma_start(wv[:], wvt_d.rearrange("(a p) m -> p a m", p=128))

            # ---------- chunk-0 q first so PE starts as soon as wq+x0 land
            def q_proj(j, xt, xoff):
                psq = ps_q.tile([C4, CHUNK], F32, tag="psq", name=f"psq{j}")
                for a in range(A):
                    nc.tensor.matmul(
                        psq[:], r(wq[:, a, :]), r(xt[:, a, ts(xoff, CHUNK)]),
                        start=(a == 0), stop=(a == A - 1),
                    )
                q_sb = qp.tile([C4, CHUNK], F32, tag="q", name=f"q{j}")
                nc.scalar.activation(r(q_sb[:]), psq[:], Ident, bias=bq_t[:])
                return q_sb

            q0 = q_proj(0, xts[0], 0)

            # ---------- k^T = WkT.T @ yT + bk : (C4, K) ----------
            ps_k = ps_q.tile([C4, K], F32, tag="psq")
            for a in range(A):
                nc.tensor.matmul(
                    ps_k[:], wk[:, a, :], yt[:, a, :],
                    start=(a == 0), stop=(a == A - 1),
                )
            kT = cp.tile([C4, K], F32)
            nc.scalar.activation(r(kT[:]), ps_k[:], Ident, bias=bk_t[:])

            # ---------- v_s = yT.T @ WvTs + ones.T @ sbv : (K, C) ----------
            ps_v = ps_t.tile([K, C], F32, tag="pst")
            for a in range(A):
                nc.tensor.matmul(
                    ps_v[:], yt[:, a, :], wv[:, a, :],
                    start=(a == 0), stop=False,
                )
            nc.tensor.matmul(ps_v[:], ones_row[:], sbv_row[:], start=False, stop=True)
            v_sb = cp.tile([K, C], F32)
            nc.vector.tensor_copy(r(v_sb[:]), ps_v[:])

            # ---------- streaming pipeline ----------
            outts = {}
            for j in range(NJ):
                d, xoff = divmod(j, DCHUNK // CHUNK)
                if d + 1 < ND and len(xts) == d + 1:
                    xts.append(load_x(d + 1))   # prefetch next DMA chunk
                xt = xts[d]

                q_sb = q0 if j == 0 else q_proj(j, xt, xoff)

                # energy = q.T @ kT : (CHUNK, K) as NS tiles of (128, K)
                pse = ps_e.tile([128, NS, K], F32, tag="pse", name=f"pse{j}")
                for s in range(NS):
                    nc.tensor.matmul(
                        pse[:, s, :], r(q_sb[:, ts(s, 128)]), r(kT[:]),
                        start=True, stop=True,
                    )

                # softmax over K (free axis); no max-subtraction needed:
                # |energy| <~ 25 so exp stays well inside fp32 range.
                exp_sb = sp.tile([128, NS, K], F32, tag="exp", name=f"exp{j}")
                sums = sp.tile([128, NS], F32, tag="sums", name=f"sums{j}")
                for s in range(NS):
                    nc.scalar.activation(
                        exp_sb[:, s, :], pse[:, s, :], Exp,
                        accum_out=sums[:, s : s + 1],
                    )
                recip = sp.tile([128, NS], F32, tag="recip", name=f"recip{j}")
                nc.vector.reciprocal(recip[:], sums[:])
                att = sp.tile([128, NS, K], F32, tag="att", name=f"att{j}")
                for s in range(NS):
                    nc.scalar.activation(
                        att[:, s, :], exp_sb[:, s, :], Ident,
                        scale=recip[:, s : s + 1],
                    )

                # att^T via PE transpose: (128, K) -> (K, 128) slices
                pst = ps_t.tile([K, CHUNK], F32, tag="pst", name=f"pst{j}")
                for s in range(NS):
                    nc.tensor.transpose(pst[:, ts(s, 128)], att[:, s, :], ident[:])
                attT = ap_.tile([K, CHUNK], F32, tag="attT", name=f"attT{j}")
                nc.scalar.copy(r(attT[:]), pst[:])

                # out = v_s.T @ att^T + x : (C, CHUNK)
                if xoff == 0:
                    outts[d] = op_.tile([128, A, DCHUNK], F32, tag="outt",
                                        name=f"outt{d}")
                outt = outts[d]
                for a in range(A):
                    pso = ps_o.tile([128, CHUNK], F32, tag="pso", name=f"pso{j}_{a}")
                    nc.tensor.matmul(
                        pso[:], r(v_sb[:, ts(a, 128)]), r(attT[:]),
                        start=True, stop=True,
                    )
                    nc.vector.tensor_add(
                        outt[:, a, ts(xoff, CHUNK)], pso[:], xt[:, a, ts(xoff, CHUNK)]
                    )
                if xoff == DCHUNK // CHUNK - 1:
                    nc.sync.dma_start(
                        out_d[:, ts(d, DCHUNK)].rearrange("(a p) q -> p a q", p=128),
                        outt[:],
                    )

    nc.compile()
    return nc


def prep_inputs(x, y, Wq, bq, Wk, bk, Wv, bv, scale):
    """Host-side prep: per-core input maps (weights transposed, scale folded)."""
    x = np.asarray(x, dtype=np.float32)
    y = np.asarray(y, dtype=np.float32)
    s = float(np.asarray(scale).reshape(-1)[0])
    shared = {
        "wqt": np.ascontiguousarray(np.asarray(Wq, np.float32).T),
        "wkt": np.ascontiguousarray(np.asarray(Wk, np.float32).T),
        "wvt": np.ascontiguousarray(np.asarray(Wv, np.float32).T * s),
        "bq": np.ascontiguousarray(np.asarray(bq, np.float32)),
        "bk": np.ascontiguousarray(np.asarray(bk, np.float32)),
        "sbv": np.ascontiguousarray(np.asarray(bv, np.float32) * s),
    }
    in_maps = []
    for n in range(N):
        in_maps.append(
            {
                "x": np.ascontiguousarray(x[n].reshape(C, HW)),
                "yt": np.ascontiguousarray(y[n].T),
                **shared,
            }
        )
    return in_maps


_NC_CACHE = {}


def get_nc(f32r: bool = True):
    if f32r not in _NC_CACHE:
        _NC_CACHE[f32r] = build(f32r)
    return _NC_CACHE[f32r]


def kernel(x, y, Wq, bq, Wk, bk, Wv, bv, scale, **run_kwargs):
    nc = get_nc()
    in_maps = prep_inputs(x, y, Wq, bq, Wk, bk, Wv, bv, scale)
    res = run_bass_kernel_spmd(nc, in_maps, core_ids=list(range(N)), **run_kwargs)
    out = np.stack([res.results[n]["out"] for n in range(N)], axis=0)
    return out.reshape(N, C, H, W).astype(np.float32)


# revision 12
# speedup vs baseline: 1.0928x; 1.0470x over previous
# CPAMDec attention decoder kernel for Trainium2 (Bass/Tile), SPMD over 8 cores.
#
# Reference computation (per batch n):
#   q = (Wq @ x_n + bq)            # (C4, HW)   1x1 conv as matmul
#   k = y_n @ Wk.T + bk            # (K, C4)
#   v = y_n @ Wv.T + bv            # (K, C)
#   energy[p,kk] = sum_m q[m,p] k[kk,m]          # (HW, K)
#   att = softmax(energy, axis=-1)
#   out_attn[c,p] = sum_kk v[kk,c] att[p,kk]     # (C, HW)
#   out = scale * out_attn + x
#
# Sharding: data-parallel over N across the 8 cores (1 batch each); weights
# replicated.  Host-side prep: weights are pre-transposed (contraction dim
# first) and `scale` is folded into Wv/bv so the device never needs it.
# bv is applied via a rank-1 PSUM accumulate onto v (ones(1,K).T @ sbv(1,C)),
# which is exact because softmax rows sum to 1.
#
# Per-core pipeline: x streams in 4 DMA chunks of (C, 1024) (4KB descriptors)
# on the sync HWDGE ring while weights ride the scalar HWDGE ring; compute
# runs on 8 sub-chunks of 512 positions:
#   q = WqT.T @ x (PE) -> +bq (ACT) -> energy (PE) -> exp (ACT) ->
#   rowsum+recip+normalize (DVE) -> att^T (PE transpose) -> copy (ACT) ->
#   U = v.T @ att^T (PE) -> out = U + x (DVE, PSUM+SBUF->SBUF) ->
#   per-(chunk, c-tile) 512KB stores.
#
# All matmuls use float32r (1 cycle/row vs fp32's 4; the PE sits at ~55%
# occupancy in this DMA-bound kernel, so HAM keeps it at 1.2 GHz — cycle
# count is what matters).  The residual path (x, +) stays exact fp32.  The
# BIR verifier requires fp32r matmul inputs to be *produced* as float32r,
# hence the bitcast views on the producing instructions (bytes unchanged for
# DMA; ACT/DVE round on write).

import numpy as np

import concourse.bacc as bacc
import concourse.mybir as mybir
import concourse.tile as tile
from concourse.bass import ts
from concourse.bass_utils import run_bass_kernel_spmd
from concourse.masks import make_identity

N, C, H, W = 8, 512, 64, 64
HW = H * W          # 4096
K = 64              # gathering centers
C4 = C // 4         # 128
A = C // 128        # 4 c-chunks of 128
DCHUNK = 1024       # hw positions per DMA chunk
ND = HW // DCHUNK   # 4
CHUNK = 512         # hw positions per compute chunk
NJ = HW // CHUNK    # 8
NS = CHUNK // 128   # 4 energy sub-chunks per compute chunk
JPD = DCHUNK // CHUNK  # compute chunks per DMA chunk

F32 = mybir.dt.float32
F32R = mybir.dt.float32r

Ident = mybir.ActivationFunctionType.Identity
Exp = mybir.ActivationFunctionType.Exp


def build(f32r: bool = True):
    nc = bacc.Bacc("TRN2", target_bir_lowering=False, debug=False)

    x_d = nc.dram_tensor("x", [C, HW], F32, kind="ExternalInput").ap()
    yt_d = nc.dram_tensor("yt", [C, K], F32, kind="ExternalInput").ap()
    wqt_d = nc.dram_tensor("wqt", [C, C4], F32, kind="ExternalInput").ap()
    wkt_d = nc.dram_tensor("wkt", [C, C4], F32, kind="ExternalInput").ap()
    wvt_d = nc.dram_tensor("wvt", [C, C], F32, kind="ExternalInput").ap()
    bq_d = nc.dram_tensor("bq", [C4], F32, kind="ExternalInput").ap()
    bk_d = nc.dram_tensor("bk", [C4], F32, kind="ExternalInput").ap()
    sbv_d = nc.dram_tensor("sbv", [C], F32, kind="ExternalInput").ap()
    out_d = nc.dram_tensor("out", [C, HW], F32, kind="ExternalOutput").ap()

    def r(ap):
        # float32r view for fp32r-matmul operands and their producers.
        return ap.bitcast(F32R) if f32r else ap

    with tile.TileContext(nc) as tc:
        with (
            tc.tile_pool(name="const", bufs=1) as cp,
            tc.tile_pool(name="xin", bufs=3) as xp,
            tc.tile_pool(name="q", bufs=3) as qp,
            tc.tile_pool(name="soft", bufs=3) as sp,
            tc.tile_pool(name="attT", bufs=3) as ap_,
            tc.tile_pool(name="osb", bufs=2) as op_,
            tc.tile_pool(name="ps_q", bufs=2, space="PSUM") as ps_q,
            tc.tile_pool(name="ps_e", bufs=1, space="PSUM") as ps_e,
            tc.tile_pool(name="ps_t", bufs=2, space="PSUM") as ps_t,
            tc.tile_pool(name="ps_o", bufs=3, space="PSUM") as ps_o,
        ):
            # ---- weights + small tensors on the scalar HWDGE ring, in
            # first-use order, so they don't serialize behind x on sync.
            wq = cp.tile([128, A, C4], F32)
            nc.scalar.dma_start(
                r(wq[:]), r(wqt_d.rearrange("(a p) m -> p a m", p=128))
            )
            bq_t = cp.tile([C4, 1], F32)
            nc.scalar.dma_start(bq_t[:], bq_d.unsqueeze(1))
            wk = cp.tile([128, A, C4], F32)
            nc.scalar.dma_start(
                r(wk[:]), r(wkt_d.rearrange("(a p) m -> p a m", p=128))
            )
            yt = cp.tile([128, A, K], F32)
            nc.scalar.dma_start(r(yt[:]), r(yt_d.rearrange("(a p) k -> p a k", p=128)))
            bk_t = cp.tile([C4, 1], F32)
            nc.scalar.dma_start(bk_t[:], bk_d.unsqueeze(1))
            wv = cp.tile([128, A, C], F32)
            nc.scalar.dma_start(
                r(wv[:]), r(wvt_d.rearrange("(a p) m -> p a m", p=128))
            )
            sbv_row = cp.tile([1, C], F32)
            nc.scalar.dma_start(sbv_row[:], sbv_d.unsqueeze(0))

            xts = []
            def load_x(d):
                xt = xp.tile([128, A, DCHUNK], F32, tag="xt", name=f"xt{d}")
                nc.sync.dma_start(
                    r(xt[:]),
                    r(x_d[:, ts(d, DCHUNK)].rearrange("(a p) q -> p a q", p=128)),
                )
                return xt
            xts.append(load_x(0))

            # pre-trigger the Exp ACT table load so it overlaps the DMAs
            warm = cp.tile([1, 1], F32)
            nc.scalar.activation(warm[:], bq_t[0:1, :], Exp)

            ones_row = cp.tile([1, K], F32)
            nc.gpsimd.memset(ones_row[:], 1.0)
            ident = cp.tile([128, 128], F32)
            make_identity(nc, ident[:])

            # ---------- chunk-0 q first so PE starts as soon as wq+x0 land
            def q_proj(j, xt, xoff):
                psq = ps_q.tile([C4, CHUNK], F32, tag="psq", name=f"psq{j}")
                for a in range(A):
                    nc.tensor.matmul(
                        psq[:], r(wq[:, a, :]), r(xt[:, a, ts(xoff, CHUNK)]),
                        start=(a == 0), stop=(a == A - 1),
                    )
                q_sb = qp.tile([C4, CHUNK], F32, tag="q", name=f"q{j}")
                nc.scalar.activation(r(q_sb[:]), psq[:], Ident, bias=bq_t[:])
                return q_sb

            q0 = q_proj(0, xts[0], 0)

            # ---------- k^T = WkT.T @ yT + bk : (C4, K) ----------
            ps_k = ps_q.tile([C4, K], F32, tag="psq")
            for a in range(A):
                nc.tensor.matmul(
                    ps_k[:], r(wk[:, a, :]), r(yt[:, a, :]),
                    start=(a == 0), stop=(a == A - 1),
                )
            kT = cp.tile([C4, K], F32)
            nc.scalar.activation(r(kT[:]), ps_k[:], Ident, bias=bk_t[:])

            # ---------- v_s = yT.T @ WvTs + ones.T @ sbv : (K, C) ----------
            ps_v = ps_t.tile([K, C], F32, tag="pst")
            for a in range(A):
                nc.tensor.matmul(
                    ps_v[:], r(yt[:, a, :]), r(wv[:, a, :]),
                    start=(a == 0), stop=False,
                )
            nc.tensor.matmul(
                ps_v[:], ones_row[:], sbv_row[:], start=False, stop=True
            )
            v_sb = cp.tile([K, C], F32)
            nc.vector.tensor_copy(r(v_sb[:]), ps_v[:])

            # ---------- streaming pipeline ----------
            outts = {}
            for j in range(NJ):
                d, xoff = divmod(j, JPD)
                if d + 1 < ND and len(xts) == d + 1:
                    xts.append(load_x(d + 1))   # prefetch next DMA chunk
                xt = xts[d]

                q_sb = q0 if j == 0 else q_proj(j, xt, xoff)

                # energy = q.T @ kT : (CHUNK, K) as NS tiles of (128, K)
                pse = ps_e.tile([128, NS, K], F32, tag="pse", name=f"pse{j}")
                for s in range(NS):
                    nc.tensor.matmul(
                        pse[:, s, :], r(q_sb[:, ts(s, 128)]), r(kT[:]),
                        start=True, stop=True,
                    )

                # softmax over K (free axis); no max-subtraction needed:
                # |energy| <~ 25 so exp stays well inside fp32 range.
                exp_sb = sp.tile([128, NS, K], F32, tag="exp", name=f"exp{j}")
                nc.scalar.activation(exp_sb[:], pse[:], Exp)
                sums = sp.tile([128, NS], F32, tag="sums", name=f"sums{j}")
                nc.vector.reduce_sum(sums[:], exp_sb[:], axis=mybir.AxisListType.X)
                recip = sp.tile([128, NS], F32, tag="recip", name=f"recip{j}")
                nc.vector.reciprocal(recip[:], sums[:])
                att = sp.tile([128, NS, K], F32, tag="att", name=f"att{j}")
                for s in range(NS):
                    nc.vector.tensor_scalar_mul(
                        att[:, s, :], exp_sb[:, s, :], recip[:, s : s + 1]
                    )

                # att^T via PE transpose: (128, K) -> (K, 128) slices
                pst = ps_t.tile([K, CHUNK], F32, tag="pst", name=f"pst{j}")
                for s in range(NS):
                    nc.tensor.transpose(pst[:, ts(s, 128)], att[:, s, :], ident[:])
                attT = ap_.tile([K, CHUNK], F32, tag="attT", name=f"attT{j}")
                nc.scalar.copy(r(attT[:]), pst[:])

                # out = v_s.T @ att^T + x : (C, CHUNK)
                if xoff == 0:
                    outts[d] = op_.tile([128, A, DCHUNK], F32, tag="outt",
                                        name=f"outt{d}")
                outt = outts[d]
                for a in range(A):
                    pso = ps_o.tile([128, CHUNK], F32, tag="pso", name=f"pso{j}_{a}")
                    nc.tensor.matmul(
                        pso[:], r(v_sb[:, ts(a, 128)]), r(attT[:]),
                        start=True, stop=True,
                    )
                    nc.vector.tensor_add(
                        outt[:, a, ts(xoff, CHUNK)], pso[:], xt[:, a, ts(xoff, CHUNK)]
                    )
                    if xoff == JPD - 1:
                        # (128, 1024) = 512KB store, 4KB per-partition lines
                        nc.sync.dma_start(
                            out_d[ts(a, 128), ts(d, DCHUNK)], outt[:, a, :]
                        )

    nc.compile()
    return nc


def prep_inputs(x, y, Wq, bq, Wk, bk, Wv, bv, scale):
    """Host-side prep: per-core input maps (weights transposed, scale folded)."""
    x = np.asarray(x, dtype=np.float32)
    y = np.asarray(y, dtype=np.float32)
    s = float(np.asarray(scale).reshape(-1)[0])
    shared = {
        "wqt": np.ascontiguousarray(np.asarray(Wq, np.float32).T),
        "wkt": np.ascontiguousarray(np.asarray(Wk, np.float32).T),
        "wvt": np.ascontiguousarray(np.asarray(Wv, np.float32).T * s),
        "bq": np.ascontiguousarray(np.asarray(bq, np.float32)),
        "bk": np.ascontiguousarray(np.asarray(bk, np.float32)),
        "sbv": np.ascontiguousarray(np.asarray(bv, np.float32) * s),
    }
    in_maps = []
    for n in range(N):
        in_maps.append(
            {
                "x": np.ascontiguousarray(x[n].reshape(C, HW)),
                "yt": np.ascontiguousarray(y[n].T),
                **shared,
            }
        )
    return in_maps


_NC_CACHE = {}


def get_nc(f32r: bool = True):
    if f32r not in _NC_CACHE:
        _NC_CACHE[f32r] = build(f32r)
    return _NC_CACHE[f32r]


def kernel(x, y, Wq, bq, Wk, bk, Wv, bv, scale, **run_kwargs):
    nc = get_nc()
    in_maps = prep_inputs(x, y, Wq, bq, Wk, bk, Wv, bv, scale)
    res = run_bass_kernel_spmd(nc, in_maps, core_ids=list(range(N)), **run_kwargs)
    out = np.stack([res.results[n]["out"] for n in range(N)], axis=0)
    return out.reshape(N, C, H, W).astype(np.float32)


# revision 13
# speedup vs baseline: 1.1617x; 1.0630x over previous
# CPAMDec attention decoder kernel for Trainium2 (Bass/Tile), SPMD over 8 cores.
#
# Reference computation (per batch n):
#   q = (Wq @ x_n + bq)            # (C4, HW)   1x1 conv as matmul
#   k = y_n @ Wk.T + bk            # (K, C4)
#   v = y_n @ Wv.T + bv            # (K, C)
#   energy[p,kk] = sum_m q[m,p] k[kk,m]          # (HW, K)
#   att = softmax(energy, axis=-1)
#   out_attn[c,p] = sum_kk v[kk,c] att[p,kk]     # (C, HW)
#   out = scale * out_attn + x
#
# Sharding: data-parallel over N across the 8 cores (1 batch each); weights
# replicated.  Host-side prep packs every weight into the exact per-partition
# SBUF byte layout (so each DMA line is 2-8KB contiguous) and folds `scale`
# into Wv/bv.  bv is applied via a rank-1 PSUM accumulate onto v
# (ones(1,K).T @ sbv(1,C)), exact because softmax rows sum to 1.
#
# Per-core pipeline: x streams in DMA chunks of (C, 1024) (4KB lines) on the
# sync HWDGE ring (first chunk split in two so compute starts early) while
# weights ride the scalar HWDGE ring; compute runs on 8 sub-chunks of 512:
#   q = WqT.T @ x (PE) -> +bq (ACT) -> energy (PE) -> exp (ACT) ->
#   rowsum+recip (DVE) -> att = exp*recip (ACT) -> att^T (PE transpose) ->
#   copy (ACT) -> U = v.T @ att^T (PE) -> out = U + x (DVE) ->
#   per-(chunk, c-tile) 512KB stores.
#
# All matmuls use float32r (1 cycle/row vs fp32's 4; the PE sits at ~55%
# occupancy in this DMA-bound kernel, so HAM keeps it at 1.2 GHz — cycle
# count is what matters).  The residual path (x, +) stays exact fp32.  The
# BIR verifier requires fp32r matmul inputs to be *produced* as float32r,
# hence the bitcast views on the producing instructions (bytes unchanged for
# DMA; ACT/DVE round on write).

import numpy as np

import concourse.bacc as bacc
import concourse.mybir as mybir
import concourse.tile as tile
from concourse.bass import ts
from concourse.bass_utils import run_bass_kernel_spmd
from concourse.masks import make_identity

N, C, H, W = 8, 512, 64, 64
HW = H * W          # 4096
K = 64              # gathering centers
C4 = C // 4         # 128
A = C // 128        # 4 c-chunks of 128
DCHUNK = 1024       # hw positions per DMA chunk
ND = HW // DCHUNK   # 4
CHUNK = 512         # hw positions per compute chunk
NJ = HW // CHUNK    # 8
NS = CHUNK // 128   # 4 energy sub-chunks per compute chunk
JPD = DCHUNK // CHUNK  # compute chunks per DMA chunk

F32 = mybir.dt.float32
F32R = mybir.dt.float32r

Ident = mybir.ActivationFunctionType.Identity
Exp = mybir.ActivationFunctionType.Exp


def build(f32r: bool = True):
    nc = bacc.Bacc("TRN2", target_bir_lowering=False, debug=False)

    x_d = nc.dram_tensor("x", [C, HW], F32, kind="ExternalInput").ap()
    # packed layouts: element [p, a*m + j] = T[a*128 + p, j] for T with
    # 128-chunked rows (see prep_inputs)
    yt_d = nc.dram_tensor("yt", [128, A * K], F32, kind="ExternalInput").ap()
    wqt_d = nc.dram_tensor("wqt", [128, A * C4], F32, kind="ExternalInput").ap()
    wkt_d = nc.dram_tensor("wkt", [128, A * C4], F32, kind="ExternalInput").ap()
    wvt_d = nc.dram_tensor("wvt", [128, A * C], F32, kind="ExternalInput").ap()
    bq_d = nc.dram_tensor("bq", [C4], F32, kind="ExternalInput").ap()
    bk_d = nc.dram_tensor("bk", [C4], F32, kind="ExternalInput").ap()
    sbv_d = nc.dram_tensor("sbv", [C], F32, kind="ExternalInput").ap()
    out_d = nc.dram_tensor("out", [C, HW], F32, kind="ExternalOutput").ap()

    def r(ap):
        # float32r view for fp32r-matmul operands and their producers.
        return ap.bitcast(F32R) if f32r else ap

    with tile.TileContext(nc) as tc:
        with (
            tc.tile_pool(name="const", bufs=1) as cp,
            tc.tile_pool(name="xin", bufs=3) as xp,
            tc.tile_pool(name="q", bufs=3) as qp,
            tc.tile_pool(name="soft", bufs=3) as sp,
            tc.tile_pool(name="attT", bufs=3) as ap_,
            tc.tile_pool(name="osb", bufs=2) as op_,
            tc.tile_pool(name="ps_q", bufs=2, space="PSUM") as ps_q,
            tc.tile_pool(name="ps_e", bufs=1, space="PSUM") as ps_e,
            tc.tile_pool(name="ps_t", bufs=2, space="PSUM") as ps_t,
            tc.tile_pool(name="ps_o", bufs=3, space="PSUM") as ps_o,
        ):
            # ---- weights + small tensors on the scalar HWDGE ring, in
            # first-use order, so they don't serialize behind x on sync.
            wq = cp.tile([128, A * C4], F32)
            nc.scalar.dma_start(r(wq[:]), r(wqt_d))
            bq_t = cp.tile([C4, 1], F32)
            nc.scalar.dma_start(bq_t[:], bq_d.unsqueeze(1))
            wk = cp.tile([128, A * C4], F32)
            nc.scalar.dma_start(r(wk[:]), r(wkt_d))
            yt = cp.tile([128, A * K], F32)
            nc.scalar.dma_start(r(yt[:]), r(yt_d))
            bk_t = cp.tile([C4, 1], F32)
            nc.scalar.dma_start(bk_t[:], bk_d.unsqueeze(1))
            sbv_row = cp.tile([1, C], F32)
            nc.scalar.dma_start(sbv_row[:], sbv_d.unsqueeze(0))
            wv = cp.tile([128, A * C], F32)
            nc.scalar.dma_start(r(wv[:]), r(wvt_d))

            xts = []
            def load_x(d, split=False):
                xt = xp.tile([128, A, DCHUNK], F32, tag="xt", name=f"xt{d}")
                # halves land (and unblock consumers) independently
                nh = 2 if split else 1
                hw_ = DCHUNK // nh
                for h in range(nh):
                    nc.sync.dma_start(
                        r(xt[:, :, ts(h, hw_)]),
                        r(
                            x_d[:, d * DCHUNK + h * hw_ : d * DCHUNK + (h + 1) * hw_]
                            .rearrange("(a p) q -> p a q", p=128)
                        ),
                    )
                return xt
            xts.append(load_x(0, split=True))

            # pre-trigger the Exp ACT table load so it overlaps the DMAs
            warm = cp.tile([1, 1], F32)
            nc.scalar.activation(warm[:], bq_t[0:1, :], Exp)

            ones_row = cp.tile([1, K], F32)
            nc.gpsimd.memset(ones_row[:], 1.0)
            ident = cp.tile([128, 128], F32)
            make_identity(nc, ident[:])

            # ---------- chunk-0 q first so PE starts as soon as wq+x0 land
            def q_proj(j, xt, xoff):
                psq = ps_q.tile([C4, CHUNK], F32, tag="psq", name=f"psq{j}")
                for a in range(A):
                    nc.tensor.matmul(
                        psq[:], r(wq[:, ts(a, C4)]), r(xt[:, a, ts(xoff, CHUNK)]),
                        start=(a == 0), stop=(a == A - 1),
                    )
                q_sb = qp.tile([C4, CHUNK], F32, tag="q", name=f"q{j}")
                nc.scalar.activation(r(q_sb[:]), psq[:], Ident, bias=bq_t[:])
                return q_sb

            q0 = q_proj(0, xts[0], 0)

            # ---------- k^T = WkT.T @ yT + bk : (C4, K) ----------
            ps_k = ps_q.tile([C4, K], F32, tag="psq")
            for a in range(A):
                nc.tensor.matmul(
                    ps_k[:], r(wk[:, ts(a, C4)]), r(yt[:, ts(a, K)]),
                    start=(a == 0), stop=(a == A - 1),
                )
            kT = cp.tile([C4, K], F32)
            nc.scalar.activation(r(kT[:]), ps_k[:], Ident, bias=bk_t[:])

            v_sb = cp.tile([K, C], F32)

            def v_setup():
                # v_s = yT.T @ WvTs + ones.T @ sbv : (K, C); emitted after
                # chunk-0 softmax so the DVE isn't head-blocked on wv/sbv.
                ps_v = ps_t.tile([K, C], F32, tag="pst")
                for a in range(A):
                    nc.tensor.matmul(
                        ps_v[:], r(yt[:, ts(a, K)]), r(wv[:, ts(a, C)]),
                        start=(a == 0), stop=False,
                    )
                nc.tensor.matmul(
                    ps_v[:], ones_row[:], sbv_row[:], start=False, stop=True
                )
                nc.vector.tensor_copy(r(v_sb[:]), ps_v[:])

            # ---------- streaming pipeline ----------
            outts = {}
            for j in range(NJ):
                d, xoff = divmod(j, JPD)
                if d + 1 < ND and len(xts) == d + 1:
                    xts.append(load_x(d + 1))   # prefetch next DMA chunk
                xt = xts[d]

                q_sb = q0 if j == 0 else q_proj(j, xt, xoff)

                # energy = q.T @ kT : (CHUNK, K) as NS tiles of (128, K)
                pse = ps_e.tile([128, NS, K], F32, tag="pse", name=f"pse{j}")
                for s in range(NS):
                    nc.tensor.matmul(
                        pse[:, s, :], r(q_sb[:, ts(s, 128)]), r(kT[:]),
                        start=True, stop=True,
                    )

                # softmax over K (free axis); no max-subtraction needed:
                # |energy| <~ 25 so exp stays well inside fp32 range.
                exp_sb = sp.tile([128, NS, K], F32, tag="exp", name=f"exp{j}")
                nc.scalar.activation(exp_sb[:], pse[:], Exp)
                sums = sp.tile([128, NS], F32, tag="sums", name=f"sums{j}")
                nc.vector.reduce_sum(sums[:], exp_sb[:], axis=mybir.AxisListType.X)
                recip = sp.tile([128, NS], F32, tag="recip", name=f"recip{j}")
                nc.vector.reciprocal(recip[:], sums[:])
                att = sp.tile([128, NS, K], F32, tag="att", name=f"att{j}")
                for s in range(NS):
                    nc.scalar.activation(
                        att[:, s, :], exp_sb[:, s, :], Ident,
                        scale=recip[:, s : s + 1],
                    )

                if j == 0:
                    v_setup()

                # att^T via PE transpose: (128, K) -> (K, 128) slices
                pst = ps_t.tile([K, CHUNK], F32, tag="pst", name=f"pst{j}")
                for s in range(NS):
                    nc.tensor.transpose(pst[:, ts(s, 128)], att[:, s, :], ident[:])
                attT = ap_.tile([K, CHUNK], F32, tag="attT", name=f"attT{j}")
                nc.scalar.copy(r(attT[:]), pst[:])

                # out = v_s.T @ att^T + x : (C, CHUNK)
                if xoff == 0:
                    outts[d] = op_.tile([128, A, DCHUNK], F32, tag="outt",
                                        name=f"outt{d}")
                outt = outts[d]
                for a in range(A):
                    pso = ps_o.tile([128, CHUNK], F32, tag="pso", name=f"pso{j}_{a}")
                    nc.tensor.matmul(
                        pso[:], r(v_sb[:, ts(a, 128)]), r(attT[:]),
                        start=True, stop=True,
                    )
                    nc.vector.tensor_add(
                        outt[:, a, ts(xoff, CHUNK)], pso[:], xt[:, a, ts(xoff, CHUNK)]
                    )
                    if xoff == JPD - 1:
                        # (128, 1024) = 512KB store, 4KB per-partition lines
                        nc.sync.dma_start(
                            out_d[ts(a, 128), ts(d, DCHUNK)], outt[:, a, :]
                        )

    nc.compile()
    return nc


def _pack_rows(t, m):
    # (A*128, m) -> (128, A*m): out[p, a*m+j] = t[a*128+p, j]
    return np.ascontiguousarray(
        t.reshape(A, 128, m).transpose(1, 0, 2).reshape(128, A * m)
    )


def prep_inputs(x, y, Wq, bq, Wk, bk, Wv, bv, scale):
    """Host-side prep: per-core input maps (weights packed, scale folded)."""
    x = np.asarray(x, dtype=np.float32)
    y = np.asarray(y, dtype=np.float32)
    s = float(np.asarray(scale).reshape(-1)[0])
    shared = {
        "wqt": _pack_rows(np.asarray(Wq, np.float32).T, C4),
        "wkt": _pack_rows(np.asarray(Wk, np.float32).T, C4),
        "wvt": _pack_rows(np.asarray(Wv, np.float32).T * s, C),
        "bq": np.ascontiguousarray(np.asarray(bq, np.float32)),
        "bk": np.ascontiguousarray(np.asarray(bk, np.float32)),
        "sbv": np.ascontiguousarray(np.asarray(bv, np.float32) * s),
    }
    in_maps = []
    for n in range(N):
        in_maps.append(
            {
                "x": np.ascontiguousarray(x[n].reshape(C, HW)),
                "yt": _pack_rows(np.ascontiguousarray(y[n].T), K),
                **shared,
            }
        )
    return in_maps


_NC_CACHE = {}


def get_nc(f32r: bool = True):
    if f32r not in _NC_CACHE:
        _NC_CACHE[f32r] = build(f32r)
    return _NC_CACHE[f32r]


def kernel(x, y, Wq, bq, Wk, bk, Wv, bv, scale, **run_kwargs):
    nc = get_nc()
    in_maps = prep_inputs(x, y, Wq, bq, Wk, bk, Wv, bv, scale)
    res = run_bass_kernel_spmd(nc, in_maps, core_ids=list(range(N)), **run_kwargs)
    out = np.stack([res.results[n]["out"] for n in range(N)], axis=0)
    return out.reshape(N, C, H, W).astype(np.float32)
